# revision 11
# baseline (speedup 1.0000x reference)
"""ConvLRUBlock Trainium2 kernel.

Reference computation (per batch b):
    h   = rms_norm(x, norm_w)                  # over channel dim
    uv  = conv3d_3x3x3(h, w_in) + b_in         # pad: replicate T/H, circular W
    u   = silu(a) * g          (a, g = uv split on channels)
    y_t = Re(h_t) c_re + Im(h_t) c_im,  h_t = lam h_{t-1} + gamma u_t  (diag LRU)
    out = x + conv3d_3x3x3(y, w_out) + b_out

Sharding: 8 cores = (batch 2) x (H quarters 4). Each core receives its H
slice plus 2 halo rows each side (edge-replicated) and the W dim circularly
padded to W+2, so no inter-core communication is needed.

In-kernel layout: channels (96) on SBUF partitions; spatial (rows x (W+2))
flattened on the free dim.

conv_in runs as bf16 matmuls with K=128 partition augmentation: partitions
96:128 of each hn plane hold a pre-shifted copy of one 32-channel group
(4 shift configs as planes), so each pass contracts one full tap (96 ch)
plus one (tap, ch-group) unit of a leftover tap: 7 passes/slab instead of 9
(42 vs 54 streamed passes per t).

conv_out runs as fp8-e4m3 DoubleRow matmuls pairing two taps per pass
(w[0]*y[tap1] + w[1]*y[tap2] per partition): 5 passes/slab instead of 9
(15 vs 27 per t). Weights are scaled by 1024 (e4m3 subnormal avoidance);
the epilogue divides via a per-partition scalar. Set out_fp8=False for a
plain bf16 conv_out (fallback if the fp8 accuracy is not acceptable).

The LRU scan is 16 sequential complex steps on the vector engine.
"""

import os
from contextlib import ExitStack

import ml_dtypes
import numpy as np

import concourse.bacc as bacc
import concourse.bass as bass  # noqa: F401
import concourse.tile as tile
from concourse import mybir

F32 = mybir.dt.float32
BF16 = mybir.dt.bfloat16
FP8 = mybir.dt.float8e4
ALU = mybir.AluOpType
AF = mybir.ActivationFunctionType
E4NP = ml_dtypes.float8_e4m3fn

EPS = 1e-6
W_SCALE = 1024.0  # fp8 weight upscale

# Full-problem constants
B_FULL, C_FULL, T_FULL, H_FULL, W_FULL = 2, 96, 16, 64, 128
QH = 4  # H quarters
N_CORES = 8

N_CFG = 4   # hn aug shift configs (planes)
N_PASS = 7  # conv_in passes per kt-slab
N_PAIR = 5  # conv_out DoubleRow passes per kt-slab


def make_schedule(Wp):
    """conv_in K=128 augmentation schedule (per kt-slab).

    Pass list entries: ((kh, kw), cfg, covered-tap-or-None). Partitions
    96:128 of hn plane `cfg` hold ch-group CFGS[cfg][0] pre-shifted by
    CFGS[cfg][1] columns, so the pass also contracts that group of the
    covered leftover tap. Leftover taps: (2,1), (2,2)."""
    cfgs = [(0, Wp), (1, 2 * Wp), (2, Wp + 1), (2, 2 * Wp + 2)]  # (grp, rho)
    passes = [
        ((0, 0), 3, (2, 2)),
        ((0, 1), 1, (2, 1)),
        ((0, 2), 1, (2, 2)),
        ((1, 0), 2, (2, 1)),
        ((1, 1), 0, (2, 1)),
        ((1, 2), 0, (2, 2)),
        ((2, 0), 0, None),  # aug slot zero-weighted
    ]
    covered = set()
    for (kh, kw), cfg, tapb in passes:
        if tapb is None:
            continue
        g, rho = cfgs[cfg]
        assert (tapb[0] * Wp + tapb[1]) - (kh * Wp + kw) == rho
        covered.add((tapb, g))
    assert len(covered) == 6
    return cfgs, passes


# conv_out DoubleRow tap pairs (within a kt-slab); deltas must be even.
OUT_PAIRS = [
    ((0, 0), (0, 2)),
    ((1, 0), (1, 2)),
    ((2, 0), (2, 2)),
    ((0, 1), (1, 1)),
    ((2, 1), None),  # second slot zero-weighted
]


def build_program(C=96, T=16, HR=16, W=128, CT=512, use_silu=True,
                  out_fp8=True):
    """Build the single-core SPMD Bass program. use_silu: Silu on ACT vs
    Sigmoid+mults (the simulator does not implement Silu)."""
    Wp = W + 2           # circular-padded width
    RIN = HR + 4         # input rows (2 halo each side, for two convs)
    RU = HR + 2          # u/y rows (1 halo each side, for conv_out)
    NIN = RIN * Wp       # flattened input cols per t
    NU = RU * Wp         # flattened u/y cols per t
    NO = HR * Wp         # flattened output cols per t
    NPL = 1 + NIN + 1    # hn plane width
    NPLU = 1 + NU + 1    # y plane width
    YDT = FP8 if out_fp8 else BF16

    CFGS, PASSES = make_schedule(Wp)

    nc = bacc.Bacc()
    xh = nc.declare_dram_parameter("xh", [C, T, RIN, Wp], F32, isOutput=False)
    w_in = nc.declare_dram_parameter("w_in", [128, 3, N_PASS, 2 * C], BF16,
                                     isOutput=False)
    if out_fp8:
        w_out = nc.declare_dram_parameter("w_out", [C, 3, N_PAIR, 2, C], FP8,
                                          isOutput=False)
    else:
        w_out = nc.declare_dram_parameter("w_out", [C, 27, C], BF16,
                                          isOutput=False)
    onesw = nc.declare_dram_parameter("onesw", [C, 128], BF16, isOutput=False)
    consts = nc.declare_dram_parameter("consts", [C, 14], F32, isOutput=False)
    out = nc.declare_dram_parameter("out", [C, T, HR, W], F32, isOutput=True)

    def col_tiles(total):
        return [(i, min(CT, total - i)) for i in range(0, total, CT)]

    with tile.TileContext(nc) as tc, ExitStack() as ctx:
        singles = ctx.enter_context(tc.tile_pool(name="singles", bufs=1))
        xpool = ctx.enter_context(tc.tile_pool(name="xpool", bufs=2))
        sqpool = ctx.enter_context(tc.tile_pool(name="sqpool", bufs=5))
        statpool = ctx.enter_context(tc.tile_pool(name="statpool", bufs=5))
        hnpool = ctx.enter_context(tc.tile_pool(name="hnpool", bufs=4))
        sapool = ctx.enter_context(tc.tile_pool(name="sapool", bufs=3))
        bpool = ctx.enter_context(tc.tile_pool(name="bpool", bufs=3))
        hrpool = ctx.enter_context(tc.tile_pool(name="hrpool", bufs=2))
        hipool = ctx.enter_context(tc.tile_pool(name="hipool", bufs=2))
        tmppool = ctx.enter_context(tc.tile_pool(name="tmppool", bufs=2))
        ypool = ctx.enter_context(tc.tile_pool(name="ypool", bufs=3))
        opool = ctx.enter_context(tc.tile_pool(name="opool", bufs=2))
        touchpool = ctx.enter_context(tc.tile_pool(name="touchpool", bufs=2))
        psN = ctx.enter_context(tc.tile_pool(name="psN", bufs=2, space="PSUM"))
        psA = ctx.enter_context(tc.tile_pool(name="psA", bufs=2, space="PSUM"))
        psG = ctx.enter_context(tc.tile_pool(name="psG", bufs=2, space="PSUM"))
        psO = ctx.enter_context(tc.tile_pool(name="psO", bufs=2, space="PSUM"))

        sb_win = singles.tile([128, 3, N_PASS, 2 * C], BF16)
        nc.sync.dma_start(out=sb_win[:], in_=w_in[:])
        if out_fp8:
            sb_wout = singles.tile([C, 3, N_PAIR, 2, C], FP8)
        else:
            sb_wout = singles.tile([C, 27, C], BF16)
        nc.sync.dma_start(out=sb_wout[:], in_=w_out[:])
        sb_ones = singles.tile([C, 128], BF16)
        nc.sync.dma_start(out=sb_ones[:], in_=onesw[:])
        sb_c = singles.tile([C, 14], F32)
        nc.sync.dma_start(out=sb_c[:], in_=consts[:])
        c_ba = sb_c[:, 0:1]
        c_bg = sb_c[:, 1:2]
        c_lr = sb_c[:, 2:3]
        c_li = sb_c[:, 3:4]
        c_nli = sb_c[:, 4:5]
        c_gcre = sb_c[:, 5:6]
        c_gcim = sb_c[:, 6:7]
        c_bout = sb_c[:, 7:8]
        c_m0 = sb_c[:, 8:9]
        c_1m0 = sb_c[:, 9:10]
        c_m1 = sb_c[:, 10:11]
        c_1m1 = sb_c[:, 11:12]
        c_eps = sb_c[:, 12:13]
        c_wsc = sb_c[:, 13:14]  # 1/W_SCALE (fp8) or 1.0

        # Warm-up reads: make each compute engine observe the const-DMA
        # semaphores early, so steady-state ops carry at most one sync wait
        # (walrus rejects DVE ops with two wait commands).
        wu_v = singles.tile([C, 14], F32)
        nc.vector.tensor_copy(wu_v[:], sb_c[:])
        wu_s = singles.tile([C, 14], F32)
        nc.scalar.activation(wu_s[:], sb_c[:], AF.Square)

        def touch(ap, engines="v"):
            """Tiny read of a freshly-DMA'd tile so the engine observes the
            DMA-queue semaphore here; later big consumers then carry only
            engine-sem waits (walrus rejects DVE ops with 2 sync waits)."""
            if "v" in engines:
                tv = touchpool.tile([C, 1], F32, tag="tv")
                nc.vector.tensor_copy(tv[:], ap)
            if "s" in engines:
                ts_ = touchpool.tile([C, 1], F32, tag="ts")
                nc.scalar.activation(ts_[:], ap, AF.Square)

        hn_slabs = [None] * T  # hnorm tiles [128, N_CFG, NPL], data at col 1
        y_slabs = [None] * T   # y tiles [C, NPLU], data at col 1

        def stage_a(t):
            """x[t] -> hnorm[t] (rms-normed, bf16, [128, N_CFG, NPL])."""
            xt = xpool.tile([C, RIN, Wp], F32, tag="xt")
            nc.sync.dma_start(out=xt[:], in_=xh[:, t])
            touch(xt[:, 0, 0:1], engines="vs")
            xf = xt[:].rearrange("p r w -> p (r w)")
            hn = hnpool.tile([128, N_CFG, NPL], BF16, tag="hn")
            for p in range(N_CFG):
                nc.vector.memset(hn[0:C, p, 0:1], 0.0)
                nc.vector.memset(hn[0:C, p, 1 + NIN:], 0.0)
            # Batch scalar ops function-wise: each activation-function switch
            # reloads the ACT table (~1.3us), so Square x5 / Ln x5 / Exp x5
            # costs 3 table loads instead of 15.
            cts = col_tiles(NIN)
            sqs, pss, lgs, invs = [], [], [], []
            for c0, n in cts:
                sq = sqpool.tile([C, CT], BF16, tag="sq")
                nc.scalar.activation(sq[:, :n], xf[:, c0:c0 + n], AF.Square)
                sqs.append(sq)
            for (c0, n), sq in zip(cts, sqs):
                ps = psN.tile([128, CT], F32, tag="psn")
                nc.tensor.matmul(ps[:, :n], sb_ones[:], sq[:, :n],
                                 start=True, stop=True)
                pss.append(ps)
            for (c0, n), ps in zip(cts, pss):
                lg = statpool.tile([C, CT], F32, tag="lg")
                nc.scalar.activation(lg[:, :n], ps[:C, :n], AF.Ln,
                                     scale=1.0 / C, bias=c_eps)
                lgs.append(lg)
            for (c0, n), lg in zip(cts, lgs):
                inv = statpool.tile([C, CT], F32, tag="inv")
                nc.scalar.activation(inv[:, :n], lg[:, :n], AF.Exp, scale=-0.5)
                invs.append(inv)
            for (c0, n), inv in zip(cts, invs):
                for p in range(N_CFG):
                    nc.vector.tensor_mul(hn[0:C, p, 1 + c0:1 + c0 + n],
                                         xf[:, c0:c0 + n], inv[:, :n])
            # partitions 96:128 of each plane: pre-shifted 32-ch group copy
            for c, (g, rho) in enumerate(CFGS):
                nc.sync.dma_start(
                    out=hn[96:128, c, 0:NPL - rho],
                    in_=hn[g * 32:(g + 1) * 32, 0, rho:NPL])
                nc.vector.memset(hn[96:128, c, NPL - rho:NPL], 0.0)
            hn_slabs[t] = hn
            return hn

        def gate_epilogue(pa, pg, bt, c0, n):
            """silu(a+ba)*(g+bg) for one coltile; a=pa[0:C], g=pg[0:C]."""
            if use_silu:
                sa = sapool.tile([C, CT], BF16, tag="sa")
                nc.scalar.activation(sa[:, :n], pa[:C, :n], AF.Silu, bias=c_ba)
            else:
                sg = sapool.tile([C, CT], BF16, tag="sg")
                nc.scalar.activation(sg[:, :n], pa[:C, :n], AF.Sigmoid,
                                     bias=c_ba)
                av = sapool.tile([C, CT], F32, tag="av")
                nc.vector.scalar_tensor_tensor(av[:, :n], pa[:C, :n], c_ba,
                                               sg[:, :n], ALU.add, ALU.bypass)
                sa = sapool.tile([C, CT], BF16, tag="sa")
                nc.vector.tensor_mul(sa[:, :n], sg[:, :n], av[:, :n])
            nc.vector.scalar_tensor_tensor(bt[:, c0:c0 + n], pg[:C, :n],
                                           c_bg, sa[:, :n],
                                           ALU.add, ALU.mult)

        scan_state = [None, None]  # hr, hi tiles [C, NU] bf16

        def conv_in_scan(t):
            """hnorm[t-1..t+1] -> u[t] -> LRU step -> y[t] bulk (per-coltile
            chunked so the tensor stream is never far ahead of y)."""
            slabs = [hn_slabs[min(max(t + kt - 1, 0), T - 1)] for kt in range(3)]
            hr_old, hi_old = scan_state
            hr_new = hrpool.tile([C, NU], BF16, tag="hr")
            hi_new = hipool.tile([C, NU], BF16, tag="hi")
            yt = ypool.tile([C, NPLU], YDT, tag="yt")
            nc.vector.memset(yt[:, 0:1], 0.0)
            nc.vector.memset(yt[:, 1 + NU:], 0.0)
            n_mm = 3 * N_PASS
            for c0, n in col_tiles(NU):
                pa = psA.tile([C, CT], F32, tag="pa")
                pg = psG.tile([C, CT], F32, tag="pg")
                for half, ps in ((0, pa), (1, pg)):
                    idx = 0
                    for kt in range(3):
                        slab = slabs[kt]
                        for p, ((kh, kw), cfg, _tapb) in enumerate(PASSES):
                            s = c0 + kh * Wp + kw
                            rhs = slab[:, cfg, s:s + n]
                            nc.tensor.matmul(
                                ps[:, :n],
                                sb_win[:, kt, p, half * C:(half + 1) * C],
                                rhs, start=(idx == 0), stop=(idx == n_mm - 1))
                            idx += 1
                bt = bpool.tile([C, CT], BF16, tag="bt")
                gate_epilogue(pa, pg, bt, 0, n)
                # LRU chunk: state update + projection for cols [c0, c0+n)
                hrs = hr_new[:, c0:c0 + n]
                his = hi_new[:, c0:c0 + n]
                if t == 0:
                    nc.vector.tensor_copy(hrs, bt[:, :n])
                    nc.vector.memset(his, 0.0)
                else:
                    t1 = tmppool.tile([C, CT], BF16, tag="tA")
                    nc.vector.scalar_tensor_tensor(
                        t1[:, :n], hi_old[:, c0:c0 + n], c_nli, bt[:, :n],
                        ALU.mult, ALU.add)
                    nc.vector.scalar_tensor_tensor(
                        hrs, hr_old[:, c0:c0 + n], c_lr, t1[:, :n],
                        ALU.mult, ALU.add)
                    t2 = tmppool.tile([C, CT], BF16, tag="tB")
                    nc.vector.scalar_tensor_tensor(
                        t2[:, :n], hi_old[:, c0:c0 + n], c_lr,
                        hi_old[:, c0:c0 + n], ALU.mult, ALU.bypass)
                    nc.vector.scalar_tensor_tensor(
                        his, hr_old[:, c0:c0 + n], c_li, t2[:, :n],
                        ALU.mult, ALU.add)
                t3 = tmppool.tile([C, CT], BF16, tag="tC")
                nc.vector.scalar_tensor_tensor(t3[:, :n], hrs, c_gcre, hrs,
                                               ALU.mult, ALU.bypass)
                nc.vector.scalar_tensor_tensor(yt[:, 1 + c0:1 + c0 + n], his,
                                               c_gcim, t3[:, :n],
                                               ALU.mult, ALU.add)
            scan_state[0], scan_state[1] = hr_new, hi_new
            # W wrap columns: col 0 <- col W (w=W-1), col W+1 <- col 1 (w=0)
            yv = yt[:, 1:1 + NU].rearrange("p (r w) -> p r w", w=Wp)
            nc.vector.tensor_copy(yv[:, :, 0:1], yv[:, :, W:W + 1])
            nc.vector.tensor_copy(yv[:, :, W + 1:W + 2], yv[:, :, 1:2])
            # H edge replication (active only on global-edge cores, via mask):
            # row0 <- m0*row0 + (1-m0)*row1 ; last <- m1*last + (1-m1)*prev
            e0 = tmppool.tile([C, Wp], F32, tag="tE")
            nc.vector.scalar_tensor_tensor(e0[:], yv[:, 1, :], c_1m0,
                                           yv[:, 1, :], ALU.mult, ALU.bypass)
            nc.vector.scalar_tensor_tensor(yv[:, 0, :], yv[:, 0, :], c_m0,
                                           e0[:], ALU.mult, ALU.add)
            e1 = tmppool.tile([C, Wp], F32, tag="tE")
            nc.vector.scalar_tensor_tensor(e1[:], yv[:, RU - 2, :], c_1m1,
                                           yv[:, RU - 2, :], ALU.mult, ALU.bypass)
            nc.vector.scalar_tensor_tensor(yv[:, RU - 1, :], yv[:, RU - 1, :],
                                           c_m1, e1[:], ALU.mult, ALU.add)
            y_slabs[t] = yt
            return yt

        def conv_out(t):
            """y[t-1..t+1] -> out[t] = x + conv(y) + b_out."""
            slabs = [y_slabs[min(max(t + kt - 1, 0), T - 1)] for kt in range(3)]
            ot = opool.tile([C, HR, Wp], F32, tag="ot")
            # residual input loaded into the output staging tile
            nc.sync.dma_start(out=ot[:], in_=xh[:, t, 2:2 + HR, :])
            touch(ot[:, 0, 0:1], engines="v")
            of = ot[:].rearrange("p r w -> p (r w)")
            if out_fp8:
                # pre-add b_out to the residual staging tile
                nc.vector.scalar_tensor_tensor(of[:, :], of[:, :], c_bout,
                                               of[:, :], ALU.add, ALU.bypass)
                n_mm = 3 * N_PAIR
                for c0, n in col_tiles(NO):
                    po = psO.tile([C, CT], F32, tag="po")
                    idx = 0
                    for kt in range(3):
                        slab = slabs[kt]
                        for p, (tap1, tap2) in enumerate(OUT_PAIRS):
                            d1 = tap1[0] * Wp + tap1[1]
                            d2 = (tap2[0] * Wp + tap2[1]) if tap2 else d1
                            rhs = slab[:, c0 + d1:c0 + d1 + n].unsqueeze(1)
                            rhs.ap[1] = (d2 - d1, 2)
                            rhs.ap[2] = (1, n)
                            nc.tensor.matmul(
                                po[:, :n], sb_wout[:, kt, p], rhs,
                                start=(idx == 0), stop=(idx == n_mm - 1),
                                perf_mode=mybir.MatmulPerfMode.DoubleRow)
                            idx += 1
                    nc.vector.scalar_tensor_tensor(of[:, c0:c0 + n],
                                                   po[:, :n], c_wsc,
                                                   of[:, c0:c0 + n],
                                                   ALU.mult, ALU.add)
            else:
                n_mm = 27
                for c0, n in col_tiles(NO):
                    po = psO.tile([C, CT], F32, tag="po")
                    idx = 0
                    for kt in range(3):
                        slab = slabs[kt]
                        for kh in range(3):
                            for kw in range(3):
                                s = c0 + kh * Wp + kw
                                nc.tensor.matmul(
                                    po[:, :n], sb_wout[:, idx % 27, :],
                                    slab[:, s:s + n],
                                    start=(idx == 0), stop=(idx == n_mm - 1))
                                idx += 1
                    nc.vector.scalar_tensor_tensor(of[:, c0:c0 + n],
                                                   po[:, :n], c_bout,
                                                   of[:, c0:c0 + n],
                                                   ALU.add, ALU.add)
            nc.sync.dma_start(out=out[:, t], in_=ot[:, :, 1:1 + W])

        # stage_a(t+2) is issued AFTER conv_in/conv_out so its rms matmuls
        # sit behind conv_out in the tensor queue (then the scalar batch has
        # a full iteration of slack before conv_in(t+2) consumes the slab).
        for t in range(min(2, T)):
            stage_a(t)
        for t in range(T):
            conv_in_scan(t)
            if t >= 1:
                conv_out(t - 1)
            if t + 2 < T:
                stage_a(t + 2)
        conv_out(T - 1)

    nc.compile()
    return nc


def prep_weight_aug(w_t, Wp, n_out):
    """Pack [C, 27, n_out] tap-major weights into the augmented layout
    [128, 3, N_PASS, n_out]: rows 0:96 = base tap A, rows 96:128 = the
    covered leftover unit's 32-channel group (or zero)."""
    C = w_t.shape[0]
    CFGS, PASSES = make_schedule(Wp)
    w_aug = np.zeros((128, 3, N_PASS, n_out), np.float32)
    for kt in range(3):
        for p, ((kh, kw), cfg, tapb) in enumerate(PASSES):
            off = kt * 9 + kh * 3 + kw
            w_aug[0:C, kt, p] = w_t[:, off]
            if tapb is not None:
                g, _rho = CFGS[cfg]
                off_b = kt * 9 + tapb[0] * 3 + tapb[1]
                w_aug[96:128, kt, p] = w_t[g * 32:(g + 1) * 32, off_b]
    return w_aug.astype(ml_dtypes.bfloat16)


def prep_weight_pairs_fp8(w_t, n_out):
    """Pack [C, 27, n_out] tap-major weights into DoubleRow pair layout
    [C, 3, N_PAIR, 2, n_out] fp8, scaled by W_SCALE."""
    C = w_t.shape[0]
    w_p = np.zeros((C, 3, N_PAIR, 2, n_out), np.float32)
    for kt in range(3):
        for p, (tap1, tap2) in enumerate(OUT_PAIRS):
            w_p[:, kt, p, 0] = w_t[:, kt * 9 + tap1[0] * 3 + tap1[1]]
            if tap2 is not None:
                w_p[:, kt, p, 1] = w_t[:, kt * 9 + tap2[0] * 3 + tap2[1]]
    return np.clip(w_p * W_SCALE, -240.0, 240.0).astype(E4NP)


def prep_core_inputs(x, norm_w, conv_in_w, conv_in_b, nu_log, theta_log,
                     c_re, c_im, conv_out_w, conv_out_b, n_qh, out_fp8=True):
    """Build per-core input maps. Cores = batch-major, then H quarters."""
    B, C, T, H, W = x.shape
    HR = H // n_qh
    Wp = W + 2

    nu = np.exp(np.asarray(nu_log, np.float64))
    theta = np.exp(np.asarray(theta_log, np.float64))
    lam_re = (np.exp(-nu) * np.cos(theta)).astype(np.float32)
    lam_im = (np.exp(-nu) * np.sin(theta)).astype(np.float32)
    gamma = np.sqrt(1.0 - np.exp(-2.0 * nu))
    gcre = (gamma * np.asarray(c_re, np.float64)).astype(np.float32)
    gcim = (gamma * np.asarray(c_im, np.float64)).astype(np.float32)

    w_in_f = np.asarray(conv_in_w, np.float32) * \
        np.asarray(norm_w, np.float32)[None, :, None, None, None]
    w_in_t = np.ascontiguousarray(
        np.transpose(w_in_f, (1, 2, 3, 4, 0)).reshape(C, 27, 2 * C))
    w_out_t = np.ascontiguousarray(
        np.transpose(np.asarray(conv_out_w, np.float32),
                     (1, 2, 3, 4, 0)).reshape(C, 27, C))
    w_in_aug = prep_weight_aug(w_in_t, Wp, 2 * C)
    if out_fp8:
        w_out_k = prep_weight_pairs_fp8(w_out_t, C)
        wsc = np.full(C, 1.0 / W_SCALE, np.float32)
    else:
        w_out_k = w_out_t.astype(ml_dtypes.bfloat16)
        wsc = np.ones(C, np.float32)
    ones = np.ones((C, 128), ml_dtypes.bfloat16)

    xp = np.concatenate([x[..., -1:], x, x[..., :1]], axis=-1)  # W circular

    in_maps = []
    for b in range(B):
        for q in range(n_qh):
            rows = np.clip(np.arange(q * HR - 2, q * HR + HR + 2), 0, H - 1)
            xh = np.ascontiguousarray(xp[b][:, :, rows, :]).astype(np.float32)
            m0 = 0.0 if q == 0 else 1.0
            m1 = 0.0 if q == n_qh - 1 else 1.0
            cvec = np.stack([
                np.asarray(conv_in_b, np.float32)[:C],
                np.asarray(conv_in_b, np.float32)[C:],
                lam_re, lam_im, -lam_im, gcre, gcim,
                np.asarray(conv_out_b, np.float32),
                np.full(C, m0, np.float32), np.full(C, 1.0 - m0, np.float32),
                np.full(C, m1, np.float32), np.full(C, 1.0 - m1, np.float32),
                np.full(C, EPS, np.float32),
                wsc,
            ], axis=1)
            in_maps.append({
                "xh": xh,
                "w_in": w_in_aug,
                "w_out": w_out_k,
                "onesw": ones,
                "consts": np.ascontiguousarray(cvec),
            })
    return in_maps


LAST_RESULT = None  # BassKernelResults of the most recent kernel() call


def kernel(x, norm_w, conv_in_w, conv_in_b, nu_log, theta_log, c_re, c_im,
           conv_out_w, conv_out_b):
    global LAST_RESULT
    from concourse.bass_utils import run_bass_kernel_spmd

    x = np.asarray(x, np.float32)
    B, C, T, H, W = x.shape
    HR = H // QH
    out_fp8 = os.environ.get("KERNEL_OUT_FP8", "1") == "1"
    in_maps = prep_core_inputs(x, norm_w, conv_in_w, conv_in_b, nu_log,
                               theta_log, c_re, c_im, conv_out_w, conv_out_b,
                               QH, out_fp8=out_fp8)
    nc = build_program(C=C, T=T, HR=HR, W=W, CT=512, out_fp8=out_fp8)
    trace = os.environ.get("KERNEL_TRACE", "") == "1"
    res = run_bass_kernel_spmd(nc, in_maps, list(range(N_CORES)), trace=trace)
    LAST_RESULT = res
    out = np.empty((B, C, T, H, W), np.float32)
    for core in range(N_CORES):
        b, q = core // QH, core % QH
        out[b, :, :, q * HR:(q + 1) * HR, :] = res.results[core]["out"]
    return out


# revision 21
# speedup vs baseline: 1.0948x; 1.0948x over previous
"""ConvLRUBlock Trainium2 kernel.

Reference computation (per batch b):
    h   = rms_norm(x, norm_w)                  # over channel dim
    uv  = conv3d_3x3x3(h, w_in) + b_in         # pad: replicate T/H, circular W
    u   = silu(a) * g          (a, g = uv split on channels)
    y_t = Re(h_t) c_re + Im(h_t) c_im,  h_t = lam h_{t-1} + gamma u_t  (diag LRU)
    out = x + conv3d_3x3x3(y, w_out) + b_out

Sharding: 8 cores = (batch 2) x (H quarters 4). Each core receives its H
slice plus 2 halo rows each side (edge-replicated) and the W dim circularly
padded to W+2, so no inter-core communication is needed.

In-kernel layout: channels (96) on SBUF partitions; spatial (rows x (W+2))
flattened on the free dim.

conv_in runs as bf16 matmuls with K=128 partition augmentation: partitions
96:128 of each hn plane hold a pre-shifted copy of one 32-channel group
(4 shift configs as planes), so each pass contracts one full tap (96 ch)
plus one (tap, ch-group) unit of a leftover tap: 7 passes/slab instead of 9
(42 vs 54 streamed passes per t).

conv_out runs as fp8-e4m3 DoubleRow matmuls pairing two taps per pass
(w[0]*y[tap1] + w[1]*y[tap2] per partition): 5 passes/slab instead of 9
(15 vs 27 per t). Weights are scaled by 1024 (e4m3 subnormal avoidance);
the epilogue divides via a per-partition scalar. Set out_fp8=False for a
plain bf16 conv_out (fallback if the fp8 accuracy is not acceptable).

The LRU scan is 16 sequential complex steps on the vector engine.
"""

import os
from contextlib import ExitStack

import ml_dtypes
import numpy as np

import concourse.bacc as bacc
import concourse.bass as bass  # noqa: F401
import concourse.tile as tile
from concourse import mybir

F32 = mybir.dt.float32
BF16 = mybir.dt.bfloat16
FP8 = mybir.dt.float8e4
ALU = mybir.AluOpType
AF = mybir.ActivationFunctionType
E4NP = ml_dtypes.float8_e4m3fn

EPS = 1e-6
W_SCALE = 1024.0  # fp8 weight upscale

# Full-problem constants
B_FULL, C_FULL, T_FULL, H_FULL, W_FULL = 2, 96, 16, 64, 128
QH = 4  # H quarters
N_CORES = 8

N_CFG = 4   # hn aug shift configs (planes)
N_PASS = 7  # conv_in passes per kt-slab
N_PAIR = 5  # conv_out DoubleRow passes per kt-slab


def make_schedule(Wp):
    """conv_in K=128 augmentation schedule (per kt-slab).

    Pass list entries: ((kh, kw), cfg, covered-tap-or-None). Partitions
    96:128 of hn plane `cfg` hold ch-group CFGS[cfg][0] pre-shifted by
    CFGS[cfg][1] columns, so the pass also contracts that group of the
    covered leftover tap. Leftover taps: (2,1), (2,2)."""
    cfgs = [(0, Wp), (1, 2 * Wp), (2, Wp + 1), (2, 2 * Wp + 2)]  # (grp, rho)
    passes = [
        ((0, 0), 3, (2, 2)),
        ((0, 1), 1, (2, 1)),
        ((0, 2), 1, (2, 2)),
        ((1, 0), 2, (2, 1)),
        ((1, 1), 0, (2, 1)),
        ((1, 2), 0, (2, 2)),
        ((2, 0), 0, None),  # aug slot zero-weighted
    ]
    covered = set()
    for (kh, kw), cfg, tapb in passes:
        if tapb is None:
            continue
        g, rho = cfgs[cfg]
        assert (tapb[0] * Wp + tapb[1]) - (kh * Wp + kw) == rho
        covered.add((tapb, g))
    assert len(covered) == 6
    return cfgs, passes


# conv_out DoubleRow tap pairs (within a kt-slab); deltas must be even.
OUT_PAIRS = [
    ((0, 0), (0, 2)),
    ((1, 0), (1, 2)),
    ((2, 0), (2, 2)),
    ((0, 1), (1, 1)),
    ((2, 1), None),  # second slot zero-weighted
]


def build_program(C=96, T=16, HR=16, W=128, CT=512, use_silu=True,
                  out_fp8=True):
    """Build the single-core SPMD Bass program. use_silu: Silu on ACT vs
    Sigmoid+mults (the simulator does not implement Silu)."""
    Wp = W + 2           # circular-padded width
    RIN = HR + 4         # input rows (2 halo each side, for two convs)
    RU = HR + 2          # u/y rows (1 halo each side, for conv_out)
    NIN = RIN * Wp       # flattened input cols per t
    NU = RU * Wp         # flattened u/y cols per t
    NO = HR * Wp         # flattened output cols per t
    NPL = 1 + NIN + 1    # hn plane width
    NPLU = 1 + NU + 1    # y plane width
    YDT = FP8 if out_fp8 else BF16

    CFGS, PASSES = make_schedule(Wp)

    nc = bacc.Bacc()
    # weight slab indices 0..2 = kt taps; 3 = kt0+kt1 merged (t=0 edge),
    # 4 = kt1+kt2 merged (t=T-1 edge) -- clamped slabs share data there.
    xh = nc.declare_dram_parameter("xh", [C, T, RIN, Wp], F32, isOutput=False)
    w_in = nc.declare_dram_parameter("w_in", [128, 5, N_PASS, 2 * C], BF16,
                                     isOutput=False)
    if out_fp8:
        w_out = nc.declare_dram_parameter("w_out", [C, 5, N_PAIR, 2, C], FP8,
                                          isOutput=False)
    else:
        w_out = nc.declare_dram_parameter("w_out", [C, 27, C], BF16,
                                          isOutput=False)
    onesw = nc.declare_dram_parameter("onesw", [C, 128], BF16, isOutput=False)
    consts = nc.declare_dram_parameter("consts", [C, 14], F32, isOutput=False)
    out = nc.declare_dram_parameter("out", [C, T, HR, W], F32, isOutput=True)

    def col_tiles(total):
        return [(i, min(CT, total - i)) for i in range(0, total, CT)]

    with tile.TileContext(nc) as tc, ExitStack() as ctx:
        singles = ctx.enter_context(tc.tile_pool(name="singles", bufs=1))
        xpool = ctx.enter_context(tc.tile_pool(name="xpool", bufs=2))
        sqpool = ctx.enter_context(tc.tile_pool(name="sqpool", bufs=5))
        statpool = ctx.enter_context(tc.tile_pool(name="statpool", bufs=5))
        hnpool = ctx.enter_context(tc.tile_pool(name="hnpool", bufs=4))
        sapool = ctx.enter_context(tc.tile_pool(name="sapool", bufs=3))
        bpool = ctx.enter_context(tc.tile_pool(name="bpool", bufs=3))
        hrpool = ctx.enter_context(tc.tile_pool(name="hrpool", bufs=2))
        hipool = ctx.enter_context(tc.tile_pool(name="hipool", bufs=2))
        tmppool = ctx.enter_context(tc.tile_pool(name="tmppool", bufs=2))
        ypool = ctx.enter_context(tc.tile_pool(name="ypool", bufs=3))
        opool = ctx.enter_context(tc.tile_pool(name="opool", bufs=2))
        touchpool = ctx.enter_context(tc.tile_pool(name="touchpool", bufs=2))
        psN = ctx.enter_context(tc.tile_pool(name="psN", bufs=2, space="PSUM"))
        psA = ctx.enter_context(tc.tile_pool(name="psA", bufs=2, space="PSUM"))
        psG = ctx.enter_context(tc.tile_pool(name="psG", bufs=2, space="PSUM"))
        psO = ctx.enter_context(tc.tile_pool(name="psO", bufs=2, space="PSUM"))

        sb_win = singles.tile([128, 5, N_PASS, 2 * C], BF16)
        nc.sync.dma_start(out=sb_win[:], in_=w_in[:])
        if out_fp8:
            sb_wout = singles.tile([C, 5, N_PAIR, 2, C], FP8)
        else:
            sb_wout = singles.tile([C, 27, C], BF16)
        nc.sync.dma_start(out=sb_wout[:], in_=w_out[:])
        sb_ones = singles.tile([C, 128], BF16)
        nc.sync.dma_start(out=sb_ones[:], in_=onesw[:])
        sb_c = singles.tile([C, 14], F32)
        nc.sync.dma_start(out=sb_c[:], in_=consts[:])
        c_ba = sb_c[:, 0:1]
        c_bg = sb_c[:, 1:2]
        c_lr = sb_c[:, 2:3]
        c_li = sb_c[:, 3:4]
        c_nli = sb_c[:, 4:5]
        c_gcre = sb_c[:, 5:6]
        c_gcim = sb_c[:, 6:7]
        c_bout = sb_c[:, 7:8]
        c_m0 = sb_c[:, 8:9]
        c_1m0 = sb_c[:, 9:10]
        c_m1 = sb_c[:, 10:11]
        c_1m1 = sb_c[:, 11:12]
        c_eps = sb_c[:, 12:13]
        c_wsc = sb_c[:, 13:14]  # 1/W_SCALE (fp8) or 1.0

        # Warm-up reads: make each compute engine observe the const-DMA
        # semaphores early, so steady-state ops carry at most one sync wait
        # (walrus rejects DVE ops with two wait commands).
        wu_v = singles.tile([C, 14], F32)
        nc.vector.tensor_copy(wu_v[:], sb_c[:])
        wu_s = singles.tile([C, 14], F32)
        nc.scalar.activation(wu_s[:], sb_c[:], AF.Square)

        def touch(ap, engines="v"):
            """Tiny read of a freshly-DMA'd tile so the engine observes the
            DMA-queue semaphore here; later big consumers then carry only
            engine-sem waits (walrus rejects DVE ops with 2 sync waits)."""
            if "v" in engines:
                tv = touchpool.tile([C, 1], F32, tag="tv")
                nc.vector.tensor_copy(tv[:], ap)
            if "s" in engines:
                ts_ = touchpool.tile([C, 1], F32, tag="ts")
                nc.scalar.activation(ts_[:], ap, AF.Square)

        hn_slabs = [None] * T  # hnorm tiles [128, N_CFG, NPL], data at col 1
        y_slabs = [None] * T   # y tiles [C, NPLU], data at col 1

        # stage_a is issued in three parts so each engine's in-order queue
        # sees work at the right position: (scalar) Square batch at iteration
        # top, (mm) rms matmuls + Ln/Exp after the first conv_in group, and
        # (tail) hn plane writes + aug copies after conv_out. Each
        # activation-function switch reloads the ACT table (~1.3us), so the
        # function-wise batching also cuts table loads.
        def stage_a_scalar(t):
            xt = xpool.tile([C, RIN, Wp], F32, tag="xt")
            nc.sync.dma_start(out=xt[:], in_=xh[:, t])
            touch(xt[:, 0, 0:1], engines="v")
            tsq = touchpool.tile([C, 1], F32, tag="ts")
            nc.scalar.activation(tsq[:], xt[:, 0, 0:1], AF.Square)
            xf = xt[:].rearrange("p r w -> p (r w)")
            cts = col_tiles(NIN)
            sqs = []
            for c0, n in cts:
                sq = sqpool.tile([C, CT], BF16, tag="sq")
                nc.scalar.activation(sq[:, :n], xf[:, c0:c0 + n], AF.Square)
                sqs.append(sq)
            return (t, xf, cts, sqs)

        def stage_a_mm(st):
            t, xf, cts, sqs = st
            pss, lgs, invs = [], [], []
            for (c0, n), sq in zip(cts, sqs):
                ps = psN.tile([128, CT], F32, tag="psn")
                nc.tensor.matmul(ps[:, :n], sb_ones[:], sq[:, :n],
                                 start=True, stop=True)
                pss.append(ps)
            for (c0, n), ps in zip(cts, pss):
                lg = statpool.tile([C, CT], F32, tag="lg")
                nc.scalar.activation(lg[:, :n], ps[:C, :n], AF.Ln,
                                     scale=1.0 / C, bias=c_eps)
                lgs.append(lg)
            for (c0, n), lg in zip(cts, lgs):
                inv = statpool.tile([C, CT], F32, tag="inv")
                nc.scalar.activation(inv[:, :n], lg[:, :n], AF.Exp, scale=-0.5)
                invs.append(inv)
            return (t, xf, cts, invs)

        def stage_a_tail(sm):
            t, xf, cts, invs = sm
            hn = hnpool.tile([128, N_CFG, NPL], BF16, tag="hn")
            for p in range(N_CFG):
                nc.vector.memset(hn[0:C, p, 0:1], 0.0)
                nc.vector.memset(hn[0:C, p, 1 + NIN:], 0.0)
            for (c0, n), inv in zip(cts, invs):
                for p in range(N_CFG):
                    nc.vector.tensor_mul(hn[0:C, p, 1 + c0:1 + c0 + n],
                                         xf[:, c0:c0 + n], inv[:, :n])
            # partitions 96:128 of each plane: pre-shifted 32-ch group copy
            for c, (g, rho) in enumerate(CFGS):
                nc.sync.dma_start(
                    out=hn[96:128, c, 0:NPL - rho],
                    in_=hn[g * 32:(g + 1) * 32, 0, rho:NPL])
                nc.vector.memset(hn[96:128, c, NPL - rho:NPL], 0.0)
            hn_slabs[t] = hn
            return hn

        def stage_a(t):
            stage_a_tail(stage_a_mm(stage_a_scalar(t)))

        def slab_plan(t, slabs_all):
            """(weight-slab-idx, activation-slab) list for conv at step t,
            merging the clamped duplicate kt slab at the T edges."""
            if t == 0:
                return [(3, slabs_all[0]), (2, slabs_all[1])]
            if t == T - 1:
                return [(0, slabs_all[t - 1]), (4, slabs_all[t])]
            return [(kt, slabs_all[t + kt - 1]) for kt in range(3)]

        def gate_epilogue(pa, pg, bt, c0, n):
            """silu(a+ba)*(g+bg) for one coltile; a=pa[0:C], g=pg[0:C]."""
            if use_silu:
                sa = sapool.tile([C, CT], BF16, tag="sa")
                nc.scalar.activation(sa[:, :n], pa[:C, :n], AF.Silu, bias=c_ba)
            else:
                sg = sapool.tile([C, CT], BF16, tag="sg")
                nc.scalar.activation(sg[:, :n], pa[:C, :n], AF.Sigmoid,
                                     bias=c_ba)
                av = sapool.tile([C, CT], F32, tag="av")
                nc.vector.scalar_tensor_tensor(av[:, :n], pa[:C, :n], c_ba,
                                               sg[:, :n], ALU.add, ALU.bypass)
                sa = sapool.tile([C, CT], BF16, tag="sa")
                nc.vector.tensor_mul(sa[:, :n], sg[:, :n], av[:, :n])
            nc.vector.scalar_tensor_tensor(bt[:, c0:c0 + n], pg[:C, :n],
                                           c_bg, sa[:, :n],
                                           ALU.add, ALU.mult)

        scan_state = [None, None]  # hr, hi tiles [C, NU] bf16

        def conv_in_scan(t, mid_cb=None):
            """hnorm[t-1..t+1] -> u[t] -> LRU step -> y[t] bulk (per-coltile
            chunked so the tensor stream is never far ahead of y). mid_cb is
            issued after the first coltile (its tensor ops then sit early in
            the queue while its scalar ops follow the first Silu)."""
            plan = slab_plan(t, hn_slabs)
            hr_old, hi_old = scan_state
            hr_new = hrpool.tile([C, NU], BF16, tag="hr")
            hi_new = hipool.tile([C, NU], BF16, tag="hi")
            yt = ypool.tile([C, NPLU], YDT, tag="yt")
            nc.vector.memset(yt[:, 0:1], 0.0)
            nc.vector.memset(yt[:, 1 + NU:], 0.0)
            n_mm = len(plan) * N_PASS
            for c0, n in col_tiles(NU):
                pa = psA.tile([C, CT], F32, tag="pa")
                pg = psG.tile([C, CT], F32, tag="pg")
                for half, ps in ((0, pa), (1, pg)):
                    idx = 0
                    for wslab, slab in plan:
                        for p, ((kh, kw), cfg, _tapb) in enumerate(PASSES):
                            s = c0 + kh * Wp + kw
                            rhs = slab[:, cfg, s:s + n]
                            nc.tensor.matmul(
                                ps[:, :n],
                                sb_win[:, wslab, p, half * C:(half + 1) * C],
                                rhs, start=(idx == 0), stop=(idx == n_mm - 1))
                            idx += 1
                bt = bpool.tile([C, CT], BF16, tag="bt")
                gate_epilogue(pa, pg, bt, 0, n)
                # LRU chunk: state update + projection for cols [c0, c0+n)
                hrs = hr_new[:, c0:c0 + n]
                his = hi_new[:, c0:c0 + n]
                if t == 0:
                    nc.vector.tensor_copy(hrs, bt[:, :n])
                    nc.vector.memset(his, 0.0)
                else:
                    t1 = tmppool.tile([C, CT], BF16, tag="tA")
                    nc.vector.scalar_tensor_tensor(
                        t1[:, :n], hi_old[:, c0:c0 + n], c_nli, bt[:, :n],
                        ALU.mult, ALU.add)
                    nc.vector.scalar_tensor_tensor(
                        hrs, hr_old[:, c0:c0 + n], c_lr, t1[:, :n],
                        ALU.mult, ALU.add)
                    t2 = tmppool.tile([C, CT], BF16, tag="tB")
                    nc.vector.scalar_tensor_tensor(
                        t2[:, :n], hi_old[:, c0:c0 + n], c_lr,
                        hi_old[:, c0:c0 + n], ALU.mult, ALU.bypass)
                    nc.vector.scalar_tensor_tensor(
                        his, hr_old[:, c0:c0 + n], c_li, t2[:, :n],
                        ALU.mult, ALU.add)
                t3 = tmppool.tile([C, CT], BF16, tag="tC")
                nc.vector.scalar_tensor_tensor(t3[:, :n], hrs, c_gcre, hrs,
                                               ALU.mult, ALU.bypass)
                nc.vector.scalar_tensor_tensor(yt[:, 1 + c0:1 + c0 + n], his,
                                               c_gcim, t3[:, :n],
                                               ALU.mult, ALU.add)
                if mid_cb is not None:
                    mid_cb()
                    mid_cb = None
            scan_state[0], scan_state[1] = hr_new, hi_new
            # W wrap columns: col 0 <- col W (w=W-1), col W+1 <- col 1 (w=0)
            yv = yt[:, 1:1 + NU].rearrange("p (r w) -> p r w", w=Wp)
            nc.vector.tensor_copy(yv[:, :, 0:1], yv[:, :, W:W + 1])
            nc.vector.tensor_copy(yv[:, :, W + 1:W + 2], yv[:, :, 1:2])
            # H edge replication (active only on global-edge cores, via mask):
            # row0 <- m0*row0 + (1-m0)*row1 ; last <- m1*last + (1-m1)*prev
            e0 = tmppool.tile([C, Wp], F32, tag="tE")
            nc.vector.scalar_tensor_tensor(e0[:], yv[:, 1, :], c_1m0,
                                           yv[:, 1, :], ALU.mult, ALU.bypass)
            nc.vector.scalar_tensor_tensor(yv[:, 0, :], yv[:, 0, :], c_m0,
                                           e0[:], ALU.mult, ALU.add)
            e1 = tmppool.tile([C, Wp], F32, tag="tE")
            nc.vector.scalar_tensor_tensor(e1[:], yv[:, RU - 2, :], c_1m1,
                                           yv[:, RU - 2, :], ALU.mult, ALU.bypass)
            nc.vector.scalar_tensor_tensor(yv[:, RU - 1, :], yv[:, RU - 1, :],
                                           c_m1, e1[:], ALU.mult, ALU.add)
            y_slabs[t] = yt
            return yt

        def conv_out(t):
            """y[t-1..t+1] -> out[t] = x + conv(y) + b_out."""
            plan = slab_plan(t, y_slabs)
            ot = opool.tile([C, HR, Wp], F32, tag="ot")
            # residual input loaded into the output staging tile
            nc.sync.dma_start(out=ot[:], in_=xh[:, t, 2:2 + HR, :])
            touch(ot[:, 0, 0:1], engines="v")
            of = ot[:].rearrange("p r w -> p (r w)")
            if out_fp8:
                # pre-add b_out to the residual staging tile
                nc.vector.scalar_tensor_tensor(of[:, :], of[:, :], c_bout,
                                               of[:, :], ALU.add, ALU.bypass)
                n_mm = len(plan) * N_PAIR
                for c0, n in col_tiles(NO):
                    po = psO.tile([C, CT], F32, tag="po")
                    idx = 0
                    for wslab, slab in plan:
                        for p, (tap1, tap2) in enumerate(OUT_PAIRS):
                            d1 = tap1[0] * Wp + tap1[1]
                            d2 = (tap2[0] * Wp + tap2[1]) if tap2 else d1
                            rhs = slab[:, c0 + d1:c0 + d1 + n].unsqueeze(1)
                            rhs.ap[1] = (d2 - d1, 2)
                            rhs.ap[2] = (1, n)
                            nc.tensor.matmul(
                                po[:, :n], sb_wout[:, wslab, p], rhs,
                                start=(idx == 0), stop=(idx == n_mm - 1),
                                perf_mode=mybir.MatmulPerfMode.DoubleRow)
                            idx += 1
                    nc.vector.scalar_tensor_tensor(of[:, c0:c0 + n],
                                                   po[:, :n], c_wsc,
                                                   of[:, c0:c0 + n],
                                                   ALU.mult, ALU.add)
            else:
                slabs = [y_slabs[min(max(t + kt - 1, 0), T - 1)]
                         for kt in range(3)]
                n_mm = 27
                for c0, n in col_tiles(NO):
                    po = psO.tile([C, CT], F32, tag="po")
                    idx = 0
                    for kt in range(3):
                        slab = slabs[kt]
                        for kh in range(3):
                            for kw in range(3):
                                s = c0 + kh * Wp + kw
                                nc.tensor.matmul(
                                    po[:, :n], sb_wout[:, idx % 27, :],
                                    slab[:, s:s + n],
                                    start=(idx == 0), stop=(idx == n_mm - 1))
                                idx += 1
                    nc.vector.scalar_tensor_tensor(of[:, c0:c0 + n],
                                                   po[:, :n], c_bout,
                                                   of[:, c0:c0 + n],
                                                   ALU.add, ALU.add)
            nc.sync.dma_start(out=out[:, t], in_=ot[:, :, 1:1 + W])

        # Issue split per iteration t: the Square batch for slab t+2 goes at
        # the top (scalar runs it while conv_in streams), the rms matmuls +
        # Ln/Exp after conv_in's first coltile (tensor reaches them ~9us in,
        # Squares done by then; Ln/Exp precede the later Silus but their
        # matmul inputs are already available), and the hn writes/aug copies
        # at the end (DVE/DMA with a full iteration of slack).
        for t in range(min(2, T)):
            stage_a(t)
        for t in range(T):
            st = stage_a_scalar(t + 2) if t + 2 < T else None
            sm = [None]

            def mid(st=st, sm=sm):
                if st is not None:
                    sm[0] = stage_a_mm(st)

            conv_in_scan(t, mid_cb=mid)
            if t >= 1:
                conv_out(t - 1)
            if sm[0] is not None:
                stage_a_tail(sm[0])
        conv_out(T - 1)

    nc.compile()
    return nc


def _add_edge_slabs(w):
    """Append merged weight slabs: idx 3 = kt0+kt1 (t=0), idx 4 = kt1+kt2
    (t=T-1), for the clamped duplicate slab at the T edges."""
    e0 = (w[:, 0] + w[:, 1])[:, None]
    e1 = (w[:, 1] + w[:, 2])[:, None]
    return np.concatenate([w, e0, e1], axis=1)


def prep_weight_aug(w_t, Wp, n_out):
    """Pack [C, 27, n_out] tap-major weights into the augmented layout
    [128, 5, N_PASS, n_out]: rows 0:96 = base tap A, rows 96:128 = the
    covered leftover unit's 32-channel group (or zero). Slabs 3/4 are the
    T-edge merges."""
    C = w_t.shape[0]
    CFGS, PASSES = make_schedule(Wp)
    w_aug = np.zeros((128, 3, N_PASS, n_out), np.float32)
    for kt in range(3):
        for p, ((kh, kw), cfg, tapb) in enumerate(PASSES):
            off = kt * 9 + kh * 3 + kw
            w_aug[0:C, kt, p] = w_t[:, off]
            if tapb is not None:
                g, _rho = CFGS[cfg]
                off_b = kt * 9 + tapb[0] * 3 + tapb[1]
                w_aug[96:128, kt, p] = w_t[g * 32:(g + 1) * 32, off_b]
    return _add_edge_slabs(w_aug).astype(ml_dtypes.bfloat16)


def prep_weight_pairs_fp8(w_t, n_out):
    """Pack [C, 27, n_out] tap-major weights into DoubleRow pair layout
    [C, 5, N_PAIR, 2, n_out] fp8, scaled by W_SCALE. Slabs 3/4 are the
    T-edge merges (summed before quantization)."""
    C = w_t.shape[0]
    w_p = np.zeros((C, 3, N_PAIR, 2, n_out), np.float32)
    for kt in range(3):
        for p, (tap1, tap2) in enumerate(OUT_PAIRS):
            w_p[:, kt, p, 0] = w_t[:, kt * 9 + tap1[0] * 3 + tap1[1]]
            if tap2 is not None:
                w_p[:, kt, p, 1] = w_t[:, kt * 9 + tap2[0] * 3 + tap2[1]]
    w_p = _add_edge_slabs(w_p)
    return np.clip(w_p * W_SCALE, -240.0, 240.0).astype(E4NP)


def prep_core_inputs(x, norm_w, conv_in_w, conv_in_b, nu_log, theta_log,
                     c_re, c_im, conv_out_w, conv_out_b, n_qh, out_fp8=True):
    """Build per-core input maps. Cores = batch-major, then H quarters."""
    B, C, T, H, W = x.shape
    HR = H // n_qh
    Wp = W + 2

    nu = np.exp(np.asarray(nu_log, np.float64))
    theta = np.exp(np.asarray(theta_log, np.float64))
    lam_re = (np.exp(-nu) * np.cos(theta)).astype(np.float32)
    lam_im = (np.exp(-nu) * np.sin(theta)).astype(np.float32)
    gamma = np.sqrt(1.0 - np.exp(-2.0 * nu))
    gcre = (gamma * np.asarray(c_re, np.float64)).astype(np.float32)
    gcim = (gamma * np.asarray(c_im, np.float64)).astype(np.float32)

    w_in_f = np.asarray(conv_in_w, np.float32) * \
        np.asarray(norm_w, np.float32)[None, :, None, None, None]
    w_in_t = np.ascontiguousarray(
        np.transpose(w_in_f, (1, 2, 3, 4, 0)).reshape(C, 27, 2 * C))
    w_out_t = np.ascontiguousarray(
        np.transpose(np.asarray(conv_out_w, np.float32),
                     (1, 2, 3, 4, 0)).reshape(C, 27, C))
    w_in_aug = prep_weight_aug(w_in_t, Wp, 2 * C)
    if out_fp8:
        w_out_k = prep_weight_pairs_fp8(w_out_t, C)
        wsc = np.full(C, 1.0 / W_SCALE, np.float32)
    else:
        w_out_k = w_out_t.astype(ml_dtypes.bfloat16)
        wsc = np.ones(C, np.float32)
    ones = np.ones((C, 128), ml_dtypes.bfloat16)

    xp = np.concatenate([x[..., -1:], x, x[..., :1]], axis=-1)  # W circular

    in_maps = []
    for b in range(B):
        for q in range(n_qh):
            rows = np.clip(np.arange(q * HR - 2, q * HR + HR + 2), 0, H - 1)
            xh = np.ascontiguousarray(xp[b][:, :, rows, :]).astype(np.float32)
            m0 = 0.0 if q == 0 else 1.0
            m1 = 0.0 if q == n_qh - 1 else 1.0
            cvec = np.stack([
                np.asarray(conv_in_b, np.float32)[:C],
                np.asarray(conv_in_b, np.float32)[C:],
                lam_re, lam_im, -lam_im, gcre, gcim,
                np.asarray(conv_out_b, np.float32),
                np.full(C, m0, np.float32), np.full(C, 1.0 - m0, np.float32),
                np.full(C, m1, np.float32), np.full(C, 1.0 - m1, np.float32),
                np.full(C, EPS, np.float32),
                wsc,
            ], axis=1)
            in_maps.append({
                "xh": xh,
                "w_in": w_in_aug,
                "w_out": w_out_k,
                "onesw": ones,
                "consts": np.ascontiguousarray(cvec),
            })
    return in_maps


LAST_RESULT = None  # BassKernelResults of the most recent kernel() call


def kernel(x, norm_w, conv_in_w, conv_in_b, nu_log, theta_log, c_re, c_im,
           conv_out_w, conv_out_b):
    global LAST_RESULT
    from concourse.bass_utils import run_bass_kernel_spmd

    x = np.asarray(x, np.float32)
    B, C, T, H, W = x.shape
    HR = H // QH
    out_fp8 = os.environ.get("KERNEL_OUT_FP8", "1") == "1"
    in_maps = prep_core_inputs(x, norm_w, conv_in_w, conv_in_b, nu_log,
                               theta_log, c_re, c_im, conv_out_w, conv_out_b,
                               QH, out_fp8=out_fp8)
    nc = build_program(C=C, T=T, HR=HR, W=W, CT=512, out_fp8=out_fp8)
    trace = os.environ.get("KERNEL_TRACE", "") == "1"
    res = run_bass_kernel_spmd(nc, in_maps, list(range(N_CORES)), trace=trace)
    LAST_RESULT = res
    out = np.empty((B, C, T, H, W), np.float32)
    for core in range(N_CORES):
        b, q = core // QH, core % QH
        out[b, :, :, q * HR:(q + 1) * HR, :] = res.results[core]["out"]
    return out
